# revision 25
# baseline (speedup 1.0000x reference)
"""Trainium2 Bass kernel for nn_AddressedStateAttention.

Reference computation (B=4, T=4096, E=1024, H=16, S=64, D=64):
    q,k,v = x@W{q,k,v}.T  (per-head RoPE on q,k, interleaved-pair rotate)
    write: ww = softmax_s(k . slot_keys / sqrt(D)); slot_state = ww^T v / T  (per b,h)
    read:  rw = softmax_s(q . slot_state / sqrt(D)); ro = rw @ slot_state
    out = ro @ Wout.T

Sharding: 8 cores = 4 batches x 2 head-groups (8 heads each).  Per-core
work is fully independent (slot_state is per (b,h)); each core emits a
partial output (its head-group's contribution to the full Wout matmul),
host sums the two partials per batch.  No collectives.

Per-core device layout notes:
  - Everything "orientation A": feature dim on partitions, tokens on the
    free axis.  x fed pre-transposed ([E, T]) and bf16.
  - RoPE applied in [d, t] layout with a stream_shuffle partition pair
    swap; the rotate sign is folded into the host-precomputed sin table.
  - write logits per head are computed chunked ([t128, s]) with k' as the
    stationary operand so the write softmax is free-dim native; slot_state
    accumulates in PSUM over all token tiles.
  - read logits are [s, t] with slot_state^T stationary; the partition-dim
    softmax denominator is formed with a ones-vector matmul, broadcast
    back with a K=1 ones matmul, and normalized with a DVE divide.
"""

import os
import sys

sys.path.insert(0, "/opt/trn_rl_repo")
sys.path.insert(0, "/opt/trn_rl_repo/concourse")

import numpy as np
import ml_dtypes

import concourse.bass as bass
import concourse.tile as tile
import concourse.mybir as mybir
from concourse import bacc
from concourse.bass_utils import run_bass_kernel_spmd

BF16 = mybir.dt.bfloat16
F32 = mybir.dt.float32
NPBF16 = ml_dtypes.bfloat16

B, T, E, H, S = 4, 4096, 1024, 16, 64
D = E // H            # 64
NH = H // 2           # 8 heads per core
DH = NH * D           # 512
TT = 512              # token tile
NTT = T // TT         # 8
P = 128
ROPE_BASE = 10000.0

SWAP_MASK = [i ^ 1 for i in range(32)]

# module-level knobs for test.py
PROFILE = False
TRACE_KW = {}
LAST_RESULT = None
DEBUG = False


def _rope_tables():
    """cos/sin in [d, t] layout, replicated to 128 partitions (2 heads),
    with the interleaved-rotate sign folded into sin."""
    inv_freq = 1.0 / (ROPE_BASE ** (np.arange(0, D, 2, dtype=np.float32) / D))
    freqs = np.arange(T, dtype=np.float32)[:, None] * inv_freq[None, :]  # [T, 32]
    emb = np.concatenate([freqs, freqs], axis=-1)  # [T, 64]
    cos = np.cos(emb).T  # [64, T]
    sin = np.sin(emb).T
    sign = np.where(np.arange(D) % 2 == 0, -1.0, 1.0).astype(np.float32)
    sinS = sin * sign[:, None]
    cosT = np.tile(cos, (2, 1)).astype(NPBF16)   # [128, T]
    sinST = np.tile(sinS, (2, 1)).astype(NPBF16)
    return cosT, sinST


def build_nc():
    nc = bacc.Bacc(trn_type="TRN2")

    xT = nc.declare_dram_parameter("xT", [E, T], BF16, isOutput=False)
    wqT = nc.declare_dram_parameter("wqT", [E, DH], BF16, isOutput=False)
    wkT = nc.declare_dram_parameter("wkT", [E, DH], BF16, isOutput=False)
    wvT = nc.declare_dram_parameter("wvT", [E, DH], BF16, isOutput=False)
    woT = nc.declare_dram_parameter("woT", [DH, E], BF16, isOutput=False)
    skT2 = nc.declare_dram_parameter("skT2", [P, DH], BF16, isOutput=False)
    cosT = nc.declare_dram_parameter("cosT", [P, T], BF16, isOutput=False)
    sinST = nc.declare_dram_parameter("sinST", [P, T], BF16, isOutput=False)
    ones64 = nc.declare_dram_parameter("ones64", [S, 1], BF16, isOutput=False)
    ones1b = nc.declare_dram_parameter("ones1b", [1, S], BF16, isOutput=False)
    id64 = nc.declare_dram_parameter("id64", [S, S], BF16, isOutput=False)
    outT = nc.declare_dram_parameter("outT", [E, T], F32, isOutput=True)
    if DEBUG:
        kdbg = nc.declare_dram_parameter("kdbg", [P, TT], F32, isOutput=True)
        wwdbg = nc.declare_dram_parameter("wwdbg", [P, S], F32, isOutput=True)
        ssdbg = nc.declare_dram_parameter("ssdbg", [S, DH], F32, isOutput=True)
        ssrldbg = nc.declare_dram_parameter("ssrldbg", [P, DH], F32, isOutput=True)
        rwdbg = nc.declare_dram_parameter("rwdbg", [S, TT], F32, isOutput=True)
        rodbg = nc.declare_dram_parameter("rodbg", [P, TT], F32, isOutput=True)

    Exp = mybir.ActivationFunctionType.Exp
    Copy = mybir.ActivationFunctionType.Copy
    mult = mybir.AluOpType.mult
    add = mybir.AluOpType.add
    amax = mybir.AluOpType.max
    divide = mybir.AluOpType.divide
    AX = mybir.AxisListType.X

    with tile.TileContext(nc) as tc:
        with (
            tc.tile_pool(name="consts", bufs=1) as consts,
            tc.tile_pool(name="xp", bufs=16) as xp,
            tc.tile_pool(name="kq", bufs=10) as kqp,
            tc.tile_pool(name="vw", bufs=10) as vwp,
            tc.tile_pool(name="sm", bufs=6) as smp,
            tc.tile_pool(name="stats", bufs=8) as stp,
            tc.tile_pool(name="rop", bufs=6) as rop,
            tc.tile_pool(name="projps", bufs=4, space="PSUM") as projps,
            tc.tile_pool(name="ssps", bufs=1, space="PSUM") as ssps,
            tc.tile_pool(name="smallps", bufs=3, space="PSUM") as smallps,
        ):
            # ---- constants into SBUF (batched 3D DMAs) ----
            wk_all = consts.tile([P, 8, DH], BF16, tag="wk")
            nc.sync.dma_start(out=wk_all, in_=wkT.rearrange("(g p) d -> p g d", p=P))
            wq_all = consts.tile([P, 8, DH], BF16, tag="wq")
            nc.sync.dma_start(out=wq_all, in_=wqT.rearrange("(g p) d -> p g d", p=P))
            wv_all = consts.tile([P, 8, DH], BF16, tag="wv")
            nc.sync.dma_start(out=wv_all, in_=wvT.rearrange("(g p) d -> p g d", p=P))
            wo_all = consts.tile([P, 4, E], BF16, tag="wo")
            nc.sync.dma_start(out=wo_all, in_=woT.rearrange("(g p) d -> p g d", p=P))
            wk_s = [wk_all[:, e, :] for e in range(8)]
            wq_s = [wq_all[:, e, :] for e in range(8)]
            wv_s = [wv_all[:, e, :] for e in range(8)]
            wo_s = [wo_all[:, g, :] for g in range(4)]
            sk_s = consts.tile([P, DH], BF16, tag="sk")
            nc.sync.dma_start(out=sk_s, in_=skT2[:, :])
            cos_s = consts.tile([P, T], BF16, tag="cos")
            nc.sync.dma_start(out=cos_s, in_=cosT[:, :])
            sin_s = consts.tile([P, T], BF16, tag="sin")
            nc.sync.dma_start(out=sin_s, in_=sinST[:, :])
            one64_s = consts.tile([S, 1], BF16, tag="one64")
            nc.sync.dma_start(out=one64_s, in_=ones64[:, :])
            one1b_s = consts.tile([1, S], BF16, tag="one1b")
            nc.sync.dma_start(out=one1b_s, in_=ones1b[:, :])
            id_s = consts.tile([S, S], BF16, tag="id64")
            nc.sync.dma_start(out=id_s, in_=id64[:, :])
            # DVE "touches" of DMA-loaded constants it will read later:
            # advances DVE's observed DMA tick so subsequent DVE consumers
            # carry no DMA wait (the TT ISA struct has a 1-wait budget).
            tch = consts.tile([P, 2], BF16, tag="tch")
            nc.vector.tensor_copy(tch[:, 0:1], cos_s[:, 0:1])
            nc.vector.tensor_copy(tch[:, 1:2], sin_s[:, 0:1])

            # persistent slot-state accumulator: [s, 8 heads * d]
            ssP = ssps.tile([S, DH], F32, tag="ss")

            def tap(dst_dram, src_ap, shape, nm):
                if not DEBUG:
                    return
                t_ = vwp.tile(shape, F32, tag="tap", name=f"tap_{nm}")
                nc.scalar.activation(t_, src_ap, Copy)
                nc.sync.dma_start(out=dst_dram[:, :], in_=t_)

            def load_x(tt):
                xs = []
                for e in range(8):
                    t_ = xp.tile([P, TT], BF16, tag="xt")
                    nc.sync.dma_start(
                        out=t_, in_=xT[e * P:(e + 1) * P, tt * TT:(tt + 1) * TT])
                    xs.append(t_)
                return xs

            def proj_A(xs, w_s, tag):
                """K/Q projection: out[j][dh128, t512] f32 PSUM, j in 0..3."""
                outs = []
                for j in range(4):
                    pj = projps.tile([P, TT], F32, tag="proj")
                    for e in range(8):
                        nc.tensor.matmul(
                            pj, w_s[e][:, j * P:(j + 1) * P], xs[e],
                            start=(e == 0), stop=(e == 7))
                    outs.append(pj)
                return outs

            # ================= WRITE PASS =================
            for tt in range(NTT):
                xs = load_x(tt)
                kP = proj_A(xs, wk_s, "k")
                kS = []
                for j, pj in enumerate(kP):
                    sh = vwp.tile([P, TT], F32, tag="shuf")
                    nc.vector.stream_shuffle(sh, pj, SWAP_MASK)
                    a = vwp.tile([P, TT], BF16, tag="ropea")
                    nc.vector.tensor_tensor(
                        out=a, in0=pj, in1=cos_s[:, tt * TT:(tt + 1) * TT], op=mult)
                    b = vwp.tile([P, TT], BF16, tag="ropeb")
                    nc.vector.tensor_tensor(
                        out=b, in0=sh, in1=sin_s[:, tt * TT:(tt + 1) * TT], op=mult)
                    o = kqp.tile([P, TT], BF16, tag="kqo")
                    nc.vector.tensor_tensor(out=o, in0=a, in1=b, op=add)
                    kS.append(o)
                if DEBUG and tt == 0:
                    tap(kdbg, kS[0], [P, TT], "k")
                # V projection, orientation B: [t128, dh512]
                vS = []
                for tc4 in range(4):
                    pv = projps.tile([P, DH], F32, tag="proj")
                    for e in range(8):
                        nc.tensor.matmul(
                            pv, xs[e][:, tc4 * P:(tc4 + 1) * P], wv_s[e],
                            start=(e == 0), stop=(e == 7))
                    v_ = vwp.tile([P, DH], BF16, tag="vs")
                    nc.scalar.activation(v_, pv, Copy)
                    vS.append(v_)
                # write logits + softmax + slot_state accumulation
                for h in range(NH):
                    base = (h % 2) * 64
                    j = h // 2
                    for tc4 in range(4):
                        wlP = smallps.tile([P, S], F32, tag="sps")
                        nc.tensor.matmul(
                            wlP,
                            kS[j][base:base + 64, tc4 * P:(tc4 + 1) * P],
                            sk_s[base:base + 64, h * S:(h + 1) * S],
                            start=True, stop=True)
                        nmx = stp.tile([P, 1], F32, tag="nmx")
                        nc.vector.tensor_reduce(nmx, wlP, AX, amax, negate=True)
                        Ew = smp.tile([P, S], BF16, tag="ew")
                        Zw = stp.tile([P, 1], F32, tag="zw")
                        nc.scalar.activation(Ew, wlP, Exp, bias=nmx, accum_out=Zw)
                        Zs = stp.tile([P, 1], F32, tag="zs")
                        nc.vector.tensor_scalar_mul(Zs, Zw, float(T))
                        Zi = stp.tile([P, 1], F32, tag="zi")
                        nc.vector.reciprocal(Zi, Zs)
                        ww = smp.tile([P, S], BF16, tag="ww")
                        nc.vector.tensor_scalar_mul(ww, Ew, Zi)
                        if DEBUG and tt == 0 and h == 0 and tc4 == 0:
                            tap(wwdbg, ww, [P, S], "ww")
                        nc.tensor.matmul(
                            ssP[:, h * S:(h + 1) * S],
                            ww,
                            vS[tc4][:, h * S:(h + 1) * S],
                            start=(tt == 0 and tc4 == 0 and h == 0),
                            stop=(tt == NTT - 1 and tc4 == 3 and h == NH - 1),
                            skip_group_check=True)

            # slot_state -> SBUF, two flavors
            ss_ro = consts.tile([S, DH], BF16, tag="ssro")   # [s, d] per head
            nc.scalar.activation(ss_ro, ssP, Copy)
            ssT_P = smallps.tile([S, DH], BF16, tag="sps")
            for h in range(NH):
                nc.tensor.transpose(
                    ssT_P[:, h * S:(h + 1) * S],
                    ss_ro[:, h * S:(h + 1) * S],
                    id_s)
            ss_rl2 = consts.tile([P, DH], BF16, tag="ssrl")  # [d, s], x2 partition halves
            nc.scalar.activation(
                ss_rl2[0:64, :], ssT_P, Copy, scale=float(D ** -0.5))
            nc.gpsimd.dma_start(out=ss_rl2[64:128, :], in_=ss_rl2[0:64, :])
            if DEBUG:
                tap(ssdbg, ss_ro, [S, DH], "ss")
                tap(ssrldbg, ss_rl2, [P, DH], "ssrl")

            # ================= READ PASS =================
            for tt in range(NTT):
                xs = load_x(tt)
                qP = proj_A(xs, wq_s, "q")
                qS = []
                for j, pj in enumerate(qP):
                    sh = vwp.tile([P, TT], F32, tag="shuf")
                    nc.vector.stream_shuffle(sh, pj, SWAP_MASK)
                    a = vwp.tile([P, TT], BF16, tag="ropea")
                    nc.vector.tensor_tensor(
                        out=a, in0=pj, in1=cos_s[:, tt * TT:(tt + 1) * TT], op=mult)
                    b = vwp.tile([P, TT], BF16, tag="ropeb")
                    nc.vector.tensor_tensor(
                        out=b, in0=sh, in1=sin_s[:, tt * TT:(tt + 1) * TT], op=mult)
                    o = kqp.tile([P, TT], BF16, tag="kqo")
                    nc.vector.tensor_tensor(out=o, in0=a, in1=b, op=add)
                    qS.append(o)
                roS = []
                for j in range(4):
                    roPair = smallps.tile([P, TT], F32, tag="sps", name=f"roP{j}")
                    for h2 in range(2):
                        h = 2 * j + h2
                        base = h2 * 64
                        rlP = smallps.tile([S, TT], F32, tag="sps")
                        nc.tensor.matmul(
                            rlP,
                            ss_rl2[base:base + 64, h * S:(h + 1) * S],
                            qS[j][base:base + 64, :],
                            start=True, stop=True)
                        Er = smp.tile([S, TT], BF16, tag="er")
                        nc.scalar.activation(Er, rlP, Exp)
                        ZrP = smallps.tile([1, TT], F32, tag="sps")
                        nc.tensor.matmul(ZrP, one64_s, Er, start=True, stop=True)
                        Zr = stp.tile([1, TT], BF16, tag="zr")
                        nc.scalar.activation(Zr, ZrP, Copy)
                        ZbP = smallps.tile([S, TT], F32, tag="sps")
                        nc.tensor.matmul(ZbP, one1b_s, Zr, start=True, stop=True)
                        Zbi = smp.tile([S, TT], F32, tag="zb")
                        nc.vector.reciprocal(Zbi, ZbP)
                        rw = smp.tile([S, TT], BF16, tag="rw")
                        nc.vector.tensor_tensor(out=rw, in0=Er, in1=Zbi, op=mult)
                        if DEBUG and tt == 0 and h == 0:
                            tap(rwdbg, rw, [S, TT], "rw")
                        nc.tensor.matmul(
                            roPair[base:base + 64, :],
                            ss_ro[:, h * S:(h + 1) * S], rw,
                            start=True, stop=True,
                            tile_position=(0, base))
                    ro_t = rop.tile([P, TT], BF16, tag="roall", name=f"roall{j}")
                    nc.scalar.activation(ro_t, roPair, Copy)
                    roS.append(ro_t)
                if DEBUG and tt == 0:
                    tap(rodbg, roS[0], [P, TT], "ro")
                # output projection
                for jt in range(8):
                    fP = projps.tile([P, TT], F32, tag="proj")
                    for dt_ in range(4):
                        nc.tensor.matmul(
                            fP, wo_s[dt_][:, jt * P:(jt + 1) * P], roS[dt_],
                            start=(dt_ == 0), stop=(dt_ == 3))
                    fS = vwp.tile([P, TT], F32, tag="fout")
                    nc.scalar.activation(fS, fP, Copy)
                    nc.sync.dma_start(
                        out=outT[jt * P:(jt + 1) * P, tt * TT:(tt + 1) * TT],
                        in_=fS)
    nc.compile()
    return nc


_NC_CACHE = None


def _get_nc():
    global _NC_CACHE
    if _NC_CACHE is None:
        _NC_CACHE = build_nc()
    return _NC_CACHE


_RUNNER = None


def _get_runner():
    """Cached jitted SPMD executor over 8 axon NeuronCores.

    Mirrors concourse.bass2jax.run_bass_via_pjrt's multi-core path but
    keeps the jitted callable (and nc) alive so repeated calls don't
    re-trace, letting test.py time back-to-back NEFF executions.
    """
    global _RUNNER
    if _RUNNER is not None:
        return _RUNNER
    import jax
    from jax.sharding import Mesh, PartitionSpec
    from jax.experimental.shard_map import shard_map
    import concourse.mybir as mb
    from concourse.bass2jax import (
        _bass_exec_p, install_neuronx_cc_hook, partition_id_tensor)

    nc = _get_nc()
    install_neuronx_cc_hook()
    partition_name = nc.partition_id_tensor.name if nc.partition_id_tensor else None
    in_names, out_names, out_avals = [], [], []
    for alloc in nc.m.functions[0].allocations:
        if not isinstance(alloc, mb.MemoryLocationSet):
            continue
        name = alloc.memorylocations[0].name
        if alloc.kind == "ExternalInput":
            if name != partition_name:
                in_names.append(name)
        elif alloc.kind == "ExternalOutput":
            shape = tuple(alloc.tensor_shape)
            dtype = mb.dt.np(alloc.dtype)
            out_avals.append(jax.core.ShapedArray(shape, dtype))
            out_names.append(name)
    n_params = len(in_names)
    n_outs = len(out_avals)
    all_names = in_names + out_names
    if partition_name is not None:
        all_names.append(partition_name)
    donate = tuple(range(n_params, n_params + n_outs))

    def _body(*args):
        operands = list(args)
        if partition_name is not None:
            operands.append(partition_id_tensor())
        outs = _bass_exec_p.bind(
            *operands,
            out_avals=tuple(out_avals),
            in_names=tuple(all_names),
            out_names=tuple(out_names),
            lowering_input_output_aliases=(),
            sim_require_finite=True,
            sim_require_nnan=True,
            nc=nc,
        )
        return tuple(outs)

    devices = jax.devices()[:8]
    mesh = Mesh(np.asarray(devices), ("core",))
    in_specs = (PartitionSpec("core"),) * (n_params + n_outs)
    out_specs = (PartitionSpec("core"),) * n_outs
    sharded = jax.jit(
        shard_map(_body, mesh=mesh, in_specs=in_specs,
                  out_specs=out_specs, check_rep=False),
        donate_argnums=donate, keep_unused=True)

    class Runner:
        fn = staticmethod(sharded)
        input_names = in_names
        output_names = out_names
        output_avals = out_avals

        @staticmethod
        def concat_inputs(in_maps):
            return [np.concatenate([np.asarray(in_maps[c][n]) for c in range(8)],
                                   axis=0) for n in in_names]

        @staticmethod
        def zeros():
            return [np.zeros((8 * a.shape[0], *a.shape[1:]), a.dtype)
                    for a in out_avals]

        @staticmethod
        def run(in_maps):
            out_arrs = sharded(*Runner.concat_inputs(in_maps), *Runner.zeros())
            return [
                {n: np.asarray(out_arrs[i]).reshape(8, *out_avals[i].shape)[c]
                 for i, n in enumerate(out_names)}
                for c in range(8)
            ]

    _RUNNER = Runner
    return _RUNNER


def make_in_maps(x, Wq, Wk, Wv, Wout, slot_keys):
    cosT, sinST = _rope_tables()
    ones64 = np.ones((S, 1), dtype=NPBF16)
    ones1b = np.ones((1, S), dtype=NPBF16)
    id64 = np.eye(S, dtype=np.float32).astype(NPBF16)
    scale = float(D ** -0.5)
    in_maps = []
    for core in range(8):
        b = core // 2
        hg = core % 2
        rs = slice(hg * DH, (hg + 1) * DH)
        sk = (slot_keys[hg * NH:(hg + 1) * NH] * scale)  # [8, 64, 64]
        # [d, s] per head packed along free dim -> [64, 512]; tile to 128
        skT = np.concatenate([sk[h].T for h in range(NH)], axis=1)
        skT2 = np.tile(skT, (2, 1)).astype(NPBF16)
        in_maps.append({
            "xT": np.ascontiguousarray(x[b].T).astype(NPBF16),
            "wqT": np.ascontiguousarray(Wq[rs].T).astype(NPBF16),
            "wkT": np.ascontiguousarray(Wk[rs].T).astype(NPBF16),
            "wvT": np.ascontiguousarray(Wv[rs].T).astype(NPBF16),
            "woT": np.ascontiguousarray(Wout[:, rs].T).astype(NPBF16),
            "skT2": skT2,
            "cosT": cosT,
            "sinST": sinST,
            "ones64": ones64,
            "ones1b": ones1b,
            "id64": id64,
        })
    return in_maps


def kernel(x, Wq, Wk, Wv, Wout, slot_keys):
    x = np.asarray(x, dtype=np.float32)
    in_maps = make_in_maps(
        x, np.asarray(Wq, np.float32), np.asarray(Wk, np.float32),
        np.asarray(Wv, np.float32), np.asarray(Wout, np.float32),
        np.asarray(slot_keys, np.float32))
    results = _get_runner().run(in_maps)
    out = np.empty((B, T, E), dtype=np.float32)
    for b in range(B):
        acc = results[2 * b]["outT"].astype(np.float32) + \
            results[2 * b + 1]["outT"].astype(np.float32)
        out[b] = acc.T
    return out


def timed_exec(in_maps, reps=10):
    """Time back-to-back NEFF executions with inputs pre-staged on device.
    Returns (list of per-call wall seconds, results of last call)."""
    import time
    import jax
    r = _get_runner()
    cin = [jax.device_put(a) for a in r.concat_inputs(in_maps)]
    for a in cin:
        a.block_until_ready()
    times = []
    out_arrs = None
    for _ in range(reps):
        zz = [jax.device_put(z) for z in r.zeros()]
        for z in zz:
            z.block_until_ready()
        t0 = time.perf_counter()
        out_arrs = r.fn(*cin, *zz)
        for o in out_arrs:
            o.block_until_ready()
        times.append(time.perf_counter() - t0)
    results = [
        {n: np.asarray(out_arrs[i]).reshape(8, *r.output_avals[i].shape)[c]
         for i, n in enumerate(r.output_names)}
        for c in range(8)
    ]
    return times, results


if __name__ == "__main__":
    nc = build_nc()
    print("built ok")
